# revision 36
# baseline (speedup 1.0000x reference)
"""Multi-head causal attention with RoPE on 8 TRN2 NeuronCores.

Problem: x[2,2048,2048] @ {Wq,Wk,Wv}ᵀ -> 16-head causal attention with RoPE
-> @ Woᵀ.  Sharding: core i handles batch i//4 and head-group i%4 (4 heads,
512 of the 2048 projection channels).  Wq/Wk/Wv are row-sliced, Wo is
column-sliced; each core emits a partial yᵀ and the host sums the 4 partials
per batch (the tensor-parallel all-reduce done at unshard time).

Device-side layout choices (all matmul operands bf16, fp32 PSUM accumulate):
  - host passes xᵀ[h,s] and Wᵀ[h,o] so every matmul contracts over the
    partition dim with zero on-chip transposes
  - scores are computed transposed, Sᵀ[k,q] = Kᵀ-chunkᵀ @ Qᵀ, so the exp'd
    attention chunk is directly the rhs the PV matmul needs
  - softmax denominator: exp'd chunks are summed 4-at-a-time on the DVE
    (bf16 tensor_adds), then one ones[128,128]-lhsT matmul per quad
    accumulates the partition sum into a [128,512] PSUM with den already
    replicated on every partition (no gpsimd broadcast, no per-chunk
    128*1*512 matmuls which measure 311ns each on HW)
  - no max-subtraction: scores are ~N(0,1) after the 1/sqrt(128) scale (fused
    into the ACT exp), so exp can't overflow fp32
  - RoPE rotate-half is done with partition-offset DVE multiplies (operands
    at different partition bases) instead of SBUF->SBUF shift DMAs
  - attention output accumulates as outᵀ[d,q], which is exactly the lhsT of
    the output projection; y leaves the chip transposed and the host fixes it
  - startup: 12 warm-up matmuls on a zeroed tile ramp the PE pstate while
    x-block-0/Wq stream in as 16 interleaved single-chunk DMAs on two queues;
    Wk/Wv prefetches are emitted mid-projection so they don't steal DMA
    bandwidth from the critical first chains
"""

import numpy as np
import ml_dtypes

import concourse.bass as bass
import concourse.tile as tile
import concourse.mybir as mybir
from concourse import bacc
from concourse.bass import ts
from concourse.bass_utils import run_bass_kernel_spmd

B, S, H = 2, 2048, 2048
HEADS, HD = 16, 128
NCORES = 8
GH = 4                 # heads per core
GO = GH * HD           # 512 projection channels per core
P = 128
SB = 512               # token-block (free dim of most matmuls)
NSB = S // SB          # 4
HC = H // P            # 16 contraction chunks of the hidden dim
NKC = S // P           # 16 key-token chunks
SCALE = float(HD) ** -0.5

BF16 = mybir.dt.bfloat16
F32 = mybir.dt.float32
EXP = mybir.ActivationFunctionType.Exp

_built = {}


def _build():
    nc = bacc.Bacc(trn_type="TRN2")

    xt = nc.dram_tensor("xt", [H, S], BF16, kind="ExternalInput")
    wqt = nc.dram_tensor("wqt", [H, GO], BF16, kind="ExternalInput")
    wkt = nc.dram_tensor("wkt", [H, GO], BF16, kind="ExternalInput")
    wvt = nc.dram_tensor("wvt", [H, GO], BF16, kind="ExternalInput")
    wot = nc.dram_tensor("wot", [GO, H], BF16, kind="ExternalInput")
    cost = nc.dram_tensor("cost", [P, S], BF16, kind="ExternalInput")
    sint = nc.dram_tensor("sint", [P, S], BF16, kind="ExternalInput")
    # trimm[k, q] = 1 where k <= q else 0: the causal keep-mask for the
    # 128x128 diagonal score blocks, applied on the DVE after exp
    trimm = nc.dram_tensor("trimm", [P, P], BF16, kind="ExternalInput")
    yt = nc.dram_tensor("yt", [H, S], F32, kind="ExternalOutput")

    xt_r = xt[:].rearrange("(hc p) s -> p hc s", p=P)
    yt_r = yt[:].rearrange("(t p) s -> p t s", p=P)

    with tile.TileContext(nc) as tc:
        with (
            tc.tile_pool(name="const", bufs=1) as const,
            tc.tile_pool(name="xstream", bufs=2) as xpool,
            tc.tile_pool(name="rope", bufs=3) as rpool,
            tc.tile_pool(name="attn", bufs=3) as apool,
            tc.tile_pool(name="dsum", bufs=4) as dpool,
            tc.tile_pool(name="soft", bufs=2) as spool,
            tc.tile_pool(name="yout", bufs=6) as ypool,
            tc.tile_pool(name="pacc", bufs=2, space="PSUM") as pacc,
            tc.tile_pool(name="pscore", bufs=3, space="PSUM") as pscore,
            tc.tile_pool(name="pout", bufs=2, space="PSUM") as pout,
            tc.tile_pool(name="pden", bufs=1, space="PSUM") as pden,
        ):
            pproj = pacc
            py = pacc

            # ---- PE warm-up: ~2.5us of matmuls on a zeroed tile so the
            # tensor engine leaves its low p-state before real data lands,
            # and the in-order PE queue has work during the DMA ramp. ----
            warm = const.tile([P, SB], BF16, tag="warm")
            nc.gpsimd.memset(warm[:], 0.0)

            def warm_mm(n):
                for _ in range(n):
                    pw = pout.tile([P, SB], F32, tag="po")
                    nc.tensor.matmul(pw[:], warm[:, 0:P], warm[:],
                                     start=True, stop=True)

            warm_mm(12)

            # ---- startup-critical DMAs: x block 0 and Wq as 16 interleaved
            # single-hc pieces on two different issue queues, matching the
            # consumption order of the first projection chain. ----
            xb0 = xpool.tile([P, HC, SB], BF16, tag="xb")
            w_q = const.tile([P, HC, GO], BF16, tag="wq")
            w_k = const.tile([P, HC, GO], BF16, tag="wk")
            wk_r = wkt[:].rearrange("(hc p) o -> p hc o", p=P)
            w_v = const.tile([P, HC, GO], BF16, tag="wv")
            wv_r = wvt[:].rearrange("(hc p) o -> p hc o", p=P)
            xt0 = xt_r[:, :, ts(0, SB)]
            wq_r = wqt[:].rearrange("(hc p) o -> p hc o", p=P)
            # DMA-config rate (~0.7us per issue per queue) is the limiter for
            # fine pieces, so spread the startup-critical stream across three
            # queues in consumption order: x block 0 as 2-hc pieces
            # alternating sync/scalar, Wq head-0 columns on gpsimd, heads
            # 1-3 afterwards (consumed strictly later), then Wk right behind.
            # startup-critical pieces, listed in chain consumption order and
            # round-robined across the three DMA-capable queues so transfers
            # proceed in parallel at full aggregate bandwidth
            def xbp(lo, w):
                return (xb0[:, lo:lo + w, :], xt0[:, lo:lo + w, :])

            def wq0p(lo, w):
                return (w_q[:, lo:lo + w, ts(0, P)],
                        wq_r[:, lo:lo + w, ts(0, P)])

            for piece, (lo, w) in enumerate(
                    [(0, 1), (1, 1), (2, 2), (4, 2), (6, 2), (8, 2),
                     (10, 2), (12, 2), (14, 2)]):
                eng = nc.sync if piece % 2 == 0 else nc.scalar
                eng.dma_start(*xbp(lo, w))
            for lo, w in ((0, 2), (2, 2), (4, 4), (8, 8)):
                nc.gpsimd.dma_start(*wq0p(lo, w))
            for h in range(1, GH):
                nc.gpsimd.dma_start(w_q[:, :, ts(h, P)], wq_r[:, :, ts(h, P)])
            for i in range(4):
                eng = nc.sync if i % 2 == 0 else nc.scalar
                eng.dma_start(w_k[:, ts(i, 4), :], wk_r[:, ts(i, 4), :])
            for i in range(2):
                eng = nc.sync if i % 2 == 0 else nc.scalar
                eng.dma_start(w_v[:, ts(i, 8), :], wv_r[:, ts(i, 8), :])
            # cos/sin and x block 1 are configured later, inside the
            # projection stream behind ACT copies, so their transfers do not
            # compete with the startup-critical pieces above
            cos_t = const.tile([P, S], BF16, tag="cos")
            sin_t = const.tile([P, S], BF16, tag="sin")
            w_o = const.tile([P, GH, H], BF16, tag="wo")
            tri_t = const.tile([P, P], BF16, tag="tri")
            ones_t = const.tile([P, P], BF16, tag="ones")
            nc.gpsimd.memset(ones_t[:], 1.0)

            q_t = const.tile([P, GH, S], BF16, tag="qt")
            k_t = const.tile([P, GH, S], BF16, tag="kt")
            v_t = const.tile([P, NKC, GO], BF16, tag="vt")
            out_t = const.tile([P, GH, S], BF16, tag="ot")

            xbs = {0: xb0}

            # ---- emission generators (interleaved to keep the in-order PE
            # queue dense while ACT/DVE run dependent work).  Each yield is
            # the approximate PE cycle cost emitted since the last yield. ----

            def proj_sb(sb, xb):
                """One token-block of Q/K (with RoPE) and V projections."""
                chain = 0
                for w_t, dest in ((w_q, q_t), (w_k, k_t)):
                    for h in range(GH):
                        ps = pproj.tile([P, SB], F32, tag="pp")
                        for hcc in range(HC):
                            nc.tensor.matmul(
                                ps[:], w_t[:, hcc, ts(h, P)], xb[:, hcc, :],
                                start=(hcc == 0), stop=(hcc == HC - 1),
                            )
                        raw = dest[:, h, ts(sb, SB)]
                        nc.scalar.copy(raw, ps[:])
                        if sb == 0 and h == 0 and w_t is w_q:
                            # paced behind the Q h0 copy on the scalar queue
                            # (must be emitted before the first RoPE read):
                            # transfers start ~13.5us, well before needed
                            nc.scalar.dma_start(cos_t[:], cost[:])
                            nc.scalar.dma_start(sin_t[:], sint[:])
                        # RoPE: rot = raw*cos + shift(raw)*sin_signed.  The
                        # half-shift crosses partitions, so it must be a DMA
                        # (engines cannot read partition-offset operands);
                        # issue it from the otherwise-idle gpsimd queue.
                        tmp = rpool.tile([P, SB], BF16, tag="sh")
                        nc.gpsimd.dma_start(tmp[0:64, :], raw[64:128, :])
                        nc.gpsimd.dma_start(tmp[64:128, :], raw[0:64, :])
                        tmp2 = rpool.tile([P, SB], BF16, tag="sp")
                        nc.vector.tensor_mul(tmp2[:], tmp[:],
                                             sin_t[:, ts(sb, SB)])
                        nc.vector.tensor_mul(raw, raw, cos_t[:, ts(sb, SB)])
                        nc.vector.tensor_add(raw, raw, tmp2[:])
                        if sb == 0 and h == 0 and w_t is w_k:
                            prefetch_x(1, nc.scalar)
                        chain += 1
                        yield 16 * SB
                for j in range(SB // P):
                    ps = pproj.tile([P, GO], F32, tag="pp")
                    for hcc in range(HC):
                        nc.tensor.matmul(
                            ps[:], xb[:, hcc, ts(j, P)], w_v[:, hcc, :],
                            start=(hcc == 0), stop=(hcc == HC - 1),
                        )
                    nc.scalar.copy(v_t[:, sb * (SB // P) + j, :], ps[:])
                    yield 16 * GO

            def attn_block(b):
                """Attention for one 512-query block; yields per k-chunk.
                PV / den matmuls are emitted a few chunks late so the PE
                never waits on that chunk's exp or quad-sum."""
                nq = b + 1
                nchunks = 4 * nq
                for h in range(GH):
                    po = pout.tile([P, SB], F32, tag="po")
                    pd = pden.tile([P, SB], F32, tag="pd")
                    pending = []

                    def flush(po=po, pd=pd, pending=pending, nchunks=nchunks,
                              nq=nq, h=h):
                        kind, a, b_, c_ = pending.pop(0)
                        if kind == 0:   # PV: (c, qlo, at_ap)
                            nc.tensor.matmul(
                                po[:, b_:], v_t[:, a, ts(h, P)], c_,
                                start=(a == 0), stop=(a == nchunks - 1),
                            )
                        else:           # den: (quad, _, qs_ap)
                            nc.tensor.matmul(
                                pd[:], ones_t[:], c_,
                                start=(a == 0), stop=(a == nq - 1),
                            )

                    for quad in range(nq):
                        diag = quad == b
                        aq = apool.tile([P, 4, SB], BF16, tag="aq")
                        if diag:
                            for j in (1, 2, 3):
                                nc.gpsimd.memset(aq[:, j, 0:P * j], 0.0)
                        q01 = q23 = None
                        for half in range(2):
                            for i in range(2):
                                jj = 2 * half + i
                                c = 4 * quad + jj
                                qlo = P * jj if diag else 0
                                n = SB - qlo
                                psc = pscore.tile([P, SB], F32, tag="ps")
                                nc.tensor.matmul(
                                    psc[:, 0:n], k_t[:, h, ts(c, P)],
                                    q_t[:, h, b * SB + qlo:(b + 1) * SB],
                                    start=True, stop=True,
                                )
                                nc.scalar.activation(
                                    aq[:, jj, qlo:], psc[:, 0:n],
                                    EXP, scale=SCALE)
                                if diag:
                                    # causal keep-mask on the 128-wide
                                    # diagonal sub-block, applied after exp
                                    nc.vector.tensor_mul(
                                        aq[:, jj, qlo:qlo + P],
                                        aq[:, jj, qlo:qlo + P], tri_t[:])
                                pending.append((0, c, qlo, aq[:, jj, qlo:]))
                                while len(pending) >= 4:
                                    flush()
                                yield 2 * n + 256
                            # bf16 chunk-sum of this half as soon as its exp
                            # is emitted (shortens the end-of-head den chain)
                            hs = dpool.tile([P, SB], BF16, tag="ds")
                            nc.vector.tensor_add(
                                hs[:], aq[:, 2 * half, :],
                                aq[:, 2 * half + 1, :])
                            if half == 0:
                                q01 = hs
                            else:
                                q23 = hs
                        # one ones-matmul (lagged via pending) partition-sums
                        # the quad into the replicated [128,512] denominator
                        nc.vector.tensor_add(q01[:], q01[:], q23[:])
                        pending.append((1, quad, 0, q01[:]))
                    while pending:
                        flush()
                    rec = spool.tile([P, SB], F32, tag="rec")
                    nc.vector.reciprocal_approx_fast(rec[:], pd[:])
                    nc.vector.tensor_mul(out_t[:, h, ts(b, SB)], po[:], rec[:])

            NT = H // P

            def outproj_block(ob):
                """Output projection of one query block; yields per matmul."""
                for nt in range(NT):
                    pyt = py.tile([P, SB], F32, tag="pp")
                    for oc in range(GH):
                        nc.tensor.matmul(
                            pyt[:], w_o[:, oc, ts(nt, P)],
                            out_t[:, oc, ts(ob, SB)],
                            start=(oc == 0), stop=(oc == GH - 1),
                        )
                    ysb = ypool.tile([P, SB], F32, tag="ysb")
                    last = ob == NSB - 1 and nt == NT - 1
                    if last:
                        # pipeline the very last tile's copy+DMA in halves to
                        # shorten the exposed end-of-kernel DMA tail
                        nc.vector.tensor_copy(ysb[:, 0:SB // 2],
                                              pyt[:, 0:SB // 2])
                        nc.gpsimd.dma_start(
                            yt_r[:, nt, ob * SB:ob * SB + SB // 2],
                            ysb[:, 0:SB // 2])
                        nc.scalar.copy(ysb[:, SB // 2:], pyt[:, SB // 2:])
                        nc.sync.dma_start(
                            yt_r[:, nt, ob * SB + SB // 2:(ob + 1) * SB],
                            ysb[:, SB // 2:])
                    elif nt % 2 == 0:
                        nc.vector.tensor_copy(ysb[:], pyt[:])
                        nc.gpsimd.dma_start(yt_r[:, nt, ts(ob, SB)], ysb[:])
                    else:
                        nc.scalar.copy(ysb[:], pyt[:])
                        nc.sync.dma_start(yt_r[:, nt, ts(ob, SB)], ysb[:])
                    yield 4 * SB

            def prefetch_x(sb, eng=None):
                xb = xpool.tile([P, HC, SB], BF16, tag="xb")
                (eng or nc.sync).dma_start(xb[:], xt_r[:, :, ts(sb, SB)])
                xbs[sb] = xb

            def drain(gen):
                for _ in gen:
                    pass

            def chain2(*gens):
                for g in gens:
                    yield from g

            def interleave(primary, filler, ratio, drain_rest=True):
                """Emit primary; after each primary quantum of w cycles,
                emit ~w*ratio cycles worth of filler quanta."""
                debt = 0.0
                for w in primary:
                    debt += (w or 0) * ratio
                    while debt > 0:
                        fw = next(filler, None)
                        if fw is None:
                            debt = 0.0
                            break
                        debt -= fw
                if drain_rest:
                    drain(filler)

            # Phase A: block-0 projections (DMA-ramp limited at the start).
            # Mask constants are tiny; Wo is deferred past the phase-B1
            # blocking prefetch so its 2MB doesn't compete with the critical
            # startup stream; x block 1 is configured inside the stream.
            nc.gpsimd.dma_start(tri_t[:], trimm[:])
            drain(proj_sb(0, xb0))

            # Phase B1: proj block 1, fill with attention block 0.  x block 2
            # reuses xb0's ring slot: its DMA un-blocks as soon as the phase-A
            # readers finish, which is exactly now — and it head-of-line
            # blocks the sync queue until then, so Wo lands right after.
            prefetch_x(2)
            nc.sync.dma_start(w_o[:], wot[:].rearrange("(oc p) n -> p oc n",
                                                       p=P))
            interleave(proj_sb(1, xbs[1]), attn_block(0), 14336 / 93000)
            # Phase B2: proj blocks 2+3, fill with attention block 1 then
            # output projection of block 0.  Same ring trick for x block 3.
            prefetch_x(3)
            interleave(chain2(proj_sb(2, xbs[2]), proj_sb(3, xbs[3])),
                       chain2(attn_block(1), outproj_block(0)),
                       65536 / 187000)
            # Phase C: attention blocks 2+3, fill with outproj blocks 1+2
            interleave(chain2(attn_block(2), attn_block(3)),
                       chain2(outproj_block(1), outproj_block(2)),
                       0.50)
            # Phase D: final output projection
            drain(outproj_block(NSB - 1))

    nc.compile()
    return nc


def _get_nc():
    if "nc" not in _built:
        _built["nc"] = _build()
    return _built["nc"]


def _host_inputs(x, Wq, Wk, Wv, Wo):
    bf = ml_dtypes.bfloat16
    inv = 1.0 / (10000.0 ** (np.arange(0, HD, 2, dtype=np.float64) / HD))
    t = np.arange(S, dtype=np.float64)
    fr = np.outer(t, inv)                       # [S, 64]
    cos = np.cos(fr)
    sin = np.sin(fr)
    cosT = np.concatenate([cos, cos], axis=1).T.astype(bf)      # [128, S]
    sinT = np.concatenate([-sin, sin], axis=1).T.astype(bf)     # signed
    a = np.arange(P)
    trimm = np.where(a[:, None] <= a[None, :], 1.0, 0.0).astype(bf)

    in_maps = []
    for core in range(NCORES):
        b, g = divmod(core, GH)
        sl = slice(GO * g, GO * (g + 1))
        in_maps.append({
            "xt": np.ascontiguousarray(x[b].T).astype(bf),
            "wqt": np.ascontiguousarray(Wq[sl, :].T).astype(bf),
            "wkt": np.ascontiguousarray(Wk[sl, :].T).astype(bf),
            "wvt": np.ascontiguousarray(Wv[sl, :].T).astype(bf),
            "wot": np.ascontiguousarray(Wo[:, sl].T).astype(bf),
            "cost": cosT.copy(),
            "sint": sinT.copy(),
            "trimm": trimm.copy(),

        })
    return in_maps


def kernel(x, Wq, Wk, Wv, Wo, _trace=False):
    x = np.asarray(x, dtype=np.float32)
    Wq = np.asarray(Wq, dtype=np.float32)
    Wk = np.asarray(Wk, dtype=np.float32)
    Wv = np.asarray(Wv, dtype=np.float32)
    Wo = np.asarray(Wo, dtype=np.float32)

    nc = _get_nc()
    in_maps = _host_inputs(x, Wq, Wk, Wv, Wo)
    res = run_bass_kernel_spmd(
        nc, in_maps, core_ids=list(range(NCORES)), trace=_trace
    )
    if _trace:
        _built["last_result"] = res

    y = np.zeros((B, S, H), dtype=np.float32)
    for core in range(NCORES):
        b = core // GH
        y[b] += res.results[core]["yt"].T
    return y
